# revision 49
# baseline (speedup 1.0000x reference)
"""Trainium2 Bass kernel for nn_DynamicMLP (3-layer LIF spiking net, T=16).

Strategy (8 NeuronCores, data-parallel over batch):
  - Shard batch 1024 -> 8 x 128. Replicate weights. Zero cross-core comms.
  - TRANSPOSED layout: [hidden on partitions (128-chunks on free), batch on
    free dim]. Weights are the stationary matmul operand, activations the
    moving one; every layer's spikes come out exactly in the next layer's
    moving-operand layout, so there are NO DMA transposes anywhere.
  - ALL matmuls are fp8 DoubleRow (0.5 cyc/row), using signed RADIX-32
    digit planes: fp8e4m3 represents every integer in [-16,16], so one
    plane carries 5 bits (vs 4 for radix-16). Each DR instr contracts one
    digit product over TWO k-chunks (pairing over k, not over digit, needs
    no plane duplication in SBUF/DMA).
  - L0 (x @ W0): x centered (x-0.5, 0.5*rowsum(W0) folded into b0) and
    digit-decomposed on HOST into 4 e4m3 planes; W0*2^4 into 4 planes.
    10 cross products (i+j<=5) -> 5 DR instr per (kpair,m). 2^t can't ride
    on e4m3 planes (range), so C0 accumulates per step in 2 PSUM groups
    (A: i+j<=4 at 2^10, B: i+j=5 at 2^20) and the decayed current lives in
    SBUF as cS_t = 2^t*(c_t-2b) = cS_{t-1} + 2^{t-14} C0A + 2^{t-24} C0B.
    End-to-end scheme error ~2^-20 (CPU-validated rel_err 0.012 < 2e-2).
  - L1/L2 (spikes @ W): W*2^4 as 4 radix-32 e4m3 planes (folds 2^4/2^-1 for
    digits 1,2 and 2^4/2^-1 for 3,4 -> spike planes s*2^t, s*2^(t-10) in
    e5m2, exact since spikes are 0/1). C1/C2 keep the cross-step PSUM
    accumulation C_t = sum 2^tau I_tau; release scale 2^-(t+9).
  - Biases: one -b*2^9 matmul injected into C1/C2 at t=0; +2b enters via
    A = v0^2 - u0 + 2b (precomputed off the critical path). L0 bias lives
    in the cS init (cS_{-1} = -b_eff).
  - L2 uses only 3 of the 4 digit products (CPU-validated margin).
  - Fused LIF elementwise spread over DVE + ACT + Pool so no engine
    exceeds ~70%: releases/spikes/STT on DVE (codegen rejects
    TensorScalarPtr on Pool), A-prebuild tensor_tensor add/sub on Pool,
    copies/squares/scaled copies on ACT. Last step (no L0 filler left)
    interleaves quarter-slices of release/spike/planes with the L1/L2
    kpi-blocks that consume them; its L0-side chain is hoisted into
    emit_rest(T-2) right after the L2 emission so it overlaps that
    step's tail.
"""
import sys

sys.path.insert(0, "/opt/trn_rl_repo")

import numpy as np
import ml_dtypes

import concourse.bacc as bacc
import concourse.tile as tile
from concourse import mybir
from concourse.bass_utils import run_bass_kernel_spmd

dt = mybir.dt
F16 = dt.float16
F32 = dt.float32
E4 = dt.float8e4
E5 = dt.float8e5
Alu = mybir.AluOpType
DR = mybir.MatmulPerfMode.DoubleRow

NCORES = 8
FULL = dict(T=16, IN=2048, H0=1024, H1=1024, OUT=512, BL=128)

NXD = 4                      # x radix-32 digit planes
NWD = 4                      # weight radix-32 digit planes (all layers)
XSC = [0, -5, 0, -5]         # stored x-plane j scale 2^XSC (j=1..4)
W0SC = [0, -5, 0, -5]        # stored W0-plane i scale
W12SC = [0, -5, -4, -9]      # stored W1/Wout-plane i scale (2^(phi_i-5i));
                             # DR weight path NaNs for e4m3 values > ~16,
                             # so keep every stored plane within +-16
# L0 products (i=w digit, j=x digit): group A accumulates at 2^10, B at 2^20
PRODS_A = [(1, 1), (1, 2), (2, 1), (2, 2)]
PRODS_B = [(1, 3), (3, 1), (2, 3), (3, 2), (1, 4), (4, 1)]
EW = 4                       # weight prescale exponent, all layers
EL12 = 9                     # L1/L2 release: c = 2^-(t+9) * C
SPB = -6                     # spike plane B scale: s*2^(t+SPB)

_BUILD_CACHE = {}
DBG = False       # add per-step debug dumps (dbg_* outputs, 2 steps)
DBG_T = (0, 1)


def build(T=16, IN=2048, H0=1024, H1=1024, OUT=512, BL=128):
    key = (T, IN, H0, H1, OUT, BL, DBG)
    if key in _BUILD_CACHE:
        return _BUILD_CACHE[key]
    KP0, KP1, KP2 = IN // 256, H0 // 256, H1 // 256
    MT0, MT1, MT2 = H0 // 128, H1 // 128, OUT // 128
    ZR = 512  # psum zero-region, fp32 elems (2KB)

    nc = bacc.Bacc("TRN2", target_bir_lowering=False, debug=False, num_devices=NCORES)

    # x digit planes: dram row (kp*128+p), inner (j, two, b): 1KB runs
    xp_d = nc.dram_tensor("xp", [T, KP0 * 128, NXD * 2 * BL], E4,
                          kind="ExternalInput")
    w0d_d = nc.dram_tensor("w0d", [KP0 * 128, NWD * 2 * H0], E4,
                           kind="ExternalInput")
    w1d_d = nc.dram_tensor("w1d", [KP1 * 128, NWD * 2 * H1], E4,
                           kind="ExternalInput")
    wod_d = nc.dram_tensor("wod", [KP2 * 128, NWD * 2 * OUT], E4,
                           kind="ExternalInput")
    # all fp16 consts+biases packed: [cpos|cneg1|cneg2|br0|br1|br2]
    CSTW = 3 * 128 + H0 + H1 + OUT
    cst_d = nc.dram_tensor("cst", [2, CSTW], F16, kind="ExternalInput")
    # out[p, (c,b)] = acc for out-neuron c*128+p, batch b (host de-permutes)
    out_d = nc.dram_tensor("out", [128, OUT // 128 * BL], F32,
                           kind="ExternalOutput")
    dbg_d = {}
    if DBG:
        for nm, h in (("cs", H0), ("v0", H0), ("s0", H0), ("v1", H1),
                      ("v2", OUT), ("b2b1", H1), ("a1", H1), ("c1", H1),
                      ("sa0", H0), ("sb0", H0)):
            dbg_d[nm] = nc.dram_tensor(
                f"dbg_{nm}", [len(DBG_T), 128, h],
                F16 if nm == "s0" else (
                    E5 if nm in ("sa0", "sb0") else F32),
                kind="ExternalOutput")

    def dbg_dump(nm, t, src):
        if DBG and t in DBG_T:
            nc.sync.dma_start(out=dbg_d[nm][DBG_T.index(t)], in_=src)

    with tile.TileContext(nc) as tc:
        with tc.tile_pool(name="w", bufs=1) as wp, \
             tc.tile_pool(name="state", bufs=1) as sp, \
             tc.tile_pool(name="xs", bufs=2) as xp, \
             tc.tile_pool(name="spk", bufs=1) as kp, \
             tc.tile_pool(name="psum", bufs=1, space="PSUM") as pp:

            # ---- resident weights ----
            KH = KP0 // 2          # w0 split in 2 half-tiles for skew filler
            w0d = [wp.tile([128, KH * NWD * 2 * H0], E4, tag=f"w0d{ci}",
                           name=f"w0d{ci}") for ci in range(2)]
            w1d = wp.tile([128, KP1 * NWD * 2 * H1], E4, tag="w1d", name="w1d")
            wod = wp.tile([128, KP2 * NWD * 2 * OUT], E4, tag="wod", name="wod")

            csts = wp.tile([2, CSTW], F16, tag="csts", name="csts")
            cpos = csts[:, 0:128]
            cneg = {1: csts[:, 128:256], 2: csts[:, 256:384]}
            b_sb = {"br0": csts[:, 384:384 + H0],
                    "br1": csts[:, 384 + H0:384 + H0 + H1],
                    "br2": csts[:, 384 + H0 + H1:384 + H0 + H1 + OUT]}

            # ---- states (free dim = (hidden-chunk, batch)) ----
            HS = {0: H0, 1: H1, 2: OUT}
            st = {}
            for l in (0, 1, 2):
                for nm in ("v0", "u0"):
                    st[(l, nm)] = sp.tile([128, HS[l]], F32, tag=f"{nm}{l}",
                                          name=f"{nm}{l}")
            vT = {0: sp.tile([128, H0], F32, tag="vT0", name="vT0"),
                  1: sp.tile([128, max(H1, OUT)], F32, tag="vT12", name="vT12")}
            vT[2] = vT[1]
            A_ = {0: sp.tile([128, H0], F32, tag="A0", name="A0"),
                  1: sp.tile([128, H1], F32, tag="A1", name="A1"),
                  2: sp.tile([128, OUT], F32, tag="A2", name="A2")}
            U_ = A_  # disjoint lifetimes: A dies at release, U born at post
            c021 = sp.tile([128, max(H0, H1)], F32, tag="c021")
            B2b = {0: sp.tile([128, H0], F32, tag="B2b0", name="B2b0"),
                   1: sp.tile([128, H1], F32, tag="B2b1", name="B2b1"),
                   2: sp.tile([128, OUT], F32, tag="B2b2", name="B2b2")}
            cS0 = sp.tile([128, H0], F32, tag="cS0", name="cS0")
            dstg = sp.tile([128, H0], F32, tag="dstg", name="dstg") if DBG \
                else None
            acc = sp.tile([128, OUT], F32, tag="acc", name="acc")
            C0A = pp.tile([128, H0], F32, tag="C0A", name="C0A")
            C0B = pp.tile([128, H0], F32, tag="C0B", name="C0B")
            C = {1: pp.tile([128, H1], F32, tag="C1", name="C1"),
                 2: pp.tile([128, OUT], F32, tag="C2", name="C2")}
            pB = pp.tile([128, 512], F32, tag="pB", name="pB")

            # ---- x loading (2 halves of KH=4 k-pairs each) ----
            x_pre = {}

            def load_x(t, cis=None):
                tiles = x_pre.setdefault(t, {})
                for ci in (cis if cis is not None else range(2)):
                    if ci in tiles:
                        continue
                    xz_t = xp.tile([128, KH * NXD * 2 * BL], E4, tag="xz",
                                   name=f"xz_t{t}_{ci}")
                    rs = ci * KH * 128
                    nc.sync.dma_start(
                        out=xz_t[:].rearrange("p (k r) -> p k r", k=KH),
                        in_=xp_d[t:t + 1, rs:rs + KH * 128, :].rearrange(
                            "o (k p) r -> p (o k) r", p=128))
                    tiles[ci] = xz_t

            def dma_w0(ci):
                # per-kpair DMAs so the first matmuls start after ~1MB
                RW = NWD * 2 * H0
                for k in range(KH):
                    rs = (ci * KH + k) * 128
                    nc.sync.dma_start(
                        out=w0d[ci][:, k * RW:(k + 1) * RW],
                        in_=w0d_d[rs:rs + 128, :])

            def dma_w1d(kps):
                for k in kps:
                    nc.sync.dma_start(
                        out=w1d[:, k * NWD * 2 * H1:(k + 1) * NWD * 2 * H1],
                        in_=w1d_d[k * 128:(k + 1) * 128, :])

            def dma_wo():
                for k in range(KP2):
                    nc.sync.dma_start(
                        out=wod[:, k * NWD * 2 * OUT:(k + 1) * NWD * 2 * OUT],
                        in_=wod_d[k * 128:(k + 1) * 128, :])

            # just-in-time DMA order (single serialized DMA resource)
            load_x(0, cis=(0,))
            dma_w0(0)
            nc.sync.dma_start(out=csts[:], in_=cst_d[:])
            load_x(0, cis=(1,))
            dma_w0(1)
            dma_w1d(range(0, 1))

            # init states + consts
            for l in (0, 1, 2):
                for nm in ("v0", "u0"):
                    nc.vector.memset(st[(l, nm)][:], 0.0)
            nc.vector.memset(c021[:], 0.021)
            nc.vector.memset(acc[:], 0.0)

            bias_of = {0: "br0", 1: "br1", 2: "br2"}

            def build_B2b():
                # B2b_l[p, (c,b)] = 2*b_l[c*128+p]: PE outer products, 4
                # chunks per pB fill, one batched copy per fill
                for l in (0, 1, 2):
                    for m0 in range(0, HS[l] // 128, 4):
                        mn = min(4, HS[l] // 128 - m0)
                        for j in range(mn):
                            m = m0 + j
                            nc.tensor.matmul(
                                pB[:, j * 128:(j + 1) * 128],
                                b_sb[bias_of[l]][:, m * 128:(m + 1) * 128],
                                cpos, start=True, stop=True,
                                skip_group_check=True)
                        nc.scalar.copy(
                            B2b[l][:, m0 * 128:(m0 + mn) * 128],
                            pB[:, :mn * 128])
                # cS0 init: cS_{-1} = -b0_eff = B2b0 * (-0.5)
                nc.vector.tensor_scalar(out=cS0[:], in0=B2b[0][:],
                                        scalar1=-0.5, scalar2=None,
                                        op0=Alu.mult)

            def inject_bias(l):
                # add -b*2^EL12 into each C[l] 128-chunk at t=0
                for m in range(HS[l] // 128):
                    nc.tensor.matmul(
                        C[l][:, m * 128:(m + 1) * 128],
                        b_sb[bias_of[l]][:, m * 128:(m + 1) * 128],
                        cneg[l], start=False, stop=False,
                        skip_group_check=True)

            # ---- L0: fp8 DR radix-32 digit products over k-pairs ----
            def emit_L0(t, cis):
                load_x(t, cis=cis)
                tiles = x_pre[t]
                for ci in cis:
                    xz_t = tiles.pop(ci)
                    if not tiles:
                        x_pre.pop(t, None)
                    wt = w0d[ci]
                    # group A first: the step's first matmuls only wait on
                    # the cS+=C0A release-stt, not the C0B one
                    for grp, C0g, prods in (("A", C0A, PRODS_A),
                                            ("B", C0B, PRODS_B)):
                        for k in range(KH):
                            for pi, (i, j) in enumerate(prods):
                                lhs = wt[:, (k * NWD + (i - 1)) * 2 * H0:
                                         (k * NWD + i) * 2 * H0].rearrange(
                                    "p (two h) -> p two h", two=2)
                                rhs = xz_t[:, (k * NXD + (j - 1)) * 2 * BL:
                                           (k * NXD + j) * 2 * BL].rearrange(
                                    "p (two b) -> p two b", two=2)
                                for m in range(MT0):
                                    first = (ci == 0 and k == 0 and pi == 0
                                             and (m * 128) % ZR == 0)
                                    last = (ci == 1 and k == KH - 1
                                            and pi == len(prods) - 1
                                            and m == MT0 - 1)
                                    nc.tensor.matmul(
                                        C0g[:, m * 128:(m + 1) * 128],
                                        lhs[:, :, m * 128:(m + 1) * 128],
                                        rhs, start=first, stop=last,
                                        perf_mode=DR, skip_group_check=True)

            # ---- L1: fp8 DR radix-32 digit products over k-pairs ----
            def emit_L1(t, kpis=None):
                sA, sB = sP_cur[0]
                for kpi in (range(KP1) if kpis is None else kpis):
                    for i in range(NWD):
                        spl = sA if i < 2 else sB
                        rhs = spl[:].rearrange("p (c b) -> p c b", b=BL)[
                            :, 2 * kpi:2 * kpi + 2, :]
                        lhs = w1d[:, (kpi * NWD + i) * 2 * H1:
                                  (kpi * NWD + i + 1) * 2 * H1].rearrange(
                            "p (two h) -> p two h", two=2)
                        for m in range(MT1):
                            first = (t == 0 and kpi == 0 and i == 0 and
                                     (m * 128) % ZR == 0)
                            last = (t == T - 1 and kpi == KP1 - 1 and
                                    i == NWD - 1 and m == MT1 - 1)
                            nc.tensor.matmul(
                                C[1][:, m * 128:(m + 1) * 128],
                                lhs[:, :, m * 128:(m + 1) * 128], rhs,
                                start=first, stop=last, perf_mode=DR,
                                skip_group_check=True)
                if t == 0 and (kpis is None or kpis[-1] == KP1 - 1):
                    inject_bias(1)

            # ---- fused LIF elementwise (layout-agnostic) ----
            def lif_pre(l, t):
                """Off-path: A = v0*v0 - u0 + B2b (ACT square + 2 DVE ops)."""
                h = HS[l]
                A = A_[l][:, :h]
                v0, u0 = st[(l, "v0")], st[(l, "u0")]
                nc.scalar.square(A, v0[:])
                nc.gpsimd.tensor_tensor(out=A, in0=A, in1=u0[:],
                                        op=Alu.subtract)
                nc.gpsimd.tensor_tensor(out=A, in0=A, in1=B2b[l][:], op=Alu.add)

            def lif_release0_part(t, off, hh):
                """One column-slice of the L0 release (3 DVE ops)."""
                for off in (off,):
                    cs = cS0[:, off:off + hh]
                    nc.vector.scalar_tensor_tensor(
                        out=cs, in0=C0A[:, off:off + hh],
                        scalar=float(2.0 ** (t - 14)),
                        in1=cs, op0=Alu.mult, op1=Alu.add)
                    nc.vector.scalar_tensor_tensor(
                        out=cs, in0=C0B[:, off:off + hh],
                        scalar=float(2.0 ** (t - 24)),
                        in1=cs, op0=Alu.mult, op1=Alu.add)
                    nc.vector.scalar_tensor_tensor(
                        out=vT[0][:, off:off + hh], in0=cs,
                        scalar=float(2.0 ** -t),
                        in1=A_[0][:, off:off + hh], op0=Alu.mult, op1=Alu.add)

            def lif_release0(t, halves=1):
                """DVE, reads PSUM: cS += 2^(t-14) C0A + 2^(t-24) C0B;
                v = cS*2^-t + A."""
                hh = H0 // halves
                for off in range(0, H0, hh):
                    lif_release0_part(t, off, hh)
                dbg_dump("cs", t, cS0[:])
                dbg_dump("v0", t, vT[0][:])

            def lif_release(l, t, halves=1, off=None, hh=None):
                """DVE, reads PSUM: v = C*2^-(t+9) + A (l in {1,2})."""
                if off is not None:
                    offs = (off,)
                else:
                    hh = HS[l] // halves
                    offs = range(0, HS[l], hh)
                for off in offs:
                    nc.vector.scalar_tensor_tensor(
                        out=vT[l][:, off:off + hh], in0=C[l][:, off:off + hh],
                        scalar=float(2.0 ** (-t - EL12)),
                        in1=A_[l][:, off:off + hh], op0=Alu.mult, op1=Alu.add)

            def lif_spike(l, t, s_out, off, hh, engine=None):
                """Spike threshold for one half. DVE when chain-critical;
                Pool for the states-only fp16 copy (off the critical path)."""
                s_scale = 1.0 if l == 2 else float(2.0 ** t)
                eng = engine or nc.vector
                eng.tensor_scalar(
                    out=s_out[:, off:off + hh], in0=vT[l][:, off:off + hh],
                    scalar1=0.5, scalar2=s_scale, op0=Alu.is_gt, op1=Alu.mult)

            def spike_planes(l, t, sA, sB, off, hh):
                """fp8e5 spike planes s*2^t and s*2^(t+SPB) straight from vT
                (two independent DVE ops -> shortest release->matmul chain)."""
                for s_out, e in ((sA, t), (sB, t + SPB)):
                    nc.vector.tensor_scalar(
                        out=s_out[:, off:off + hh],
                        in0=vT[l][:, off:off + hh],
                        scalar1=0.5, scalar2=float(2.0 ** e),
                        op0=Alu.is_gt, op1=Alu.mult)

            def lif_states(l, t, s_out, last):
                """State updates for step t+1 (off critical path)."""
                h = HS[l]
                v = vT[l][:, :h]
                v0, u0 = st[(l, "v0")], st[(l, "u0")]
                s_scale = 1.0 if l == 2 else float(2.0 ** t)
                if last:
                    return
                U = U_[l][:, :h]
                nc.vector.scalar_tensor_tensor(
                    out=U, in0=v0[:], scalar=float(-0.172 / 1.529), in1=u0[:],
                    op0=Alu.mult, op1=Alu.add)
                nc.scalar.mul(U, U, 1.529)
                nc.vector.scalar_tensor_tensor(
                    out=u0[:], in0=s_out[:], scalar=float(0.132 / s_scale),
                    in1=U, op0=Alu.mult, op1=Alu.add)
                nc.scalar.copy(v0[:], v)
                nc.vector.copy_predicated(out=v0[:],
                                          mask=s_out[:].bitcast(dt.uint16),
                                          data=c021[:, :h])

            def make_planes_half(s, sA, sB, off, hh):
                """2 fp8e5 scaled copies of one half of the spikes: s*2^t
                (s already carries 2^t) and s*2^(t-10)."""
                nc.scalar.copy(sA[:, off:off + hh], s[:, off:off + hh])
                nc.scalar.mul(sB[:, off:off + hh], s[:, off:off + hh],
                              float(2.0 ** SPB))

            sP_cur = [None]
            fin0 = [False]   # final-step l0 chain already emitted?

            def emit_final_l0_chain():
                """Quarter-interleaved release0/spike/planes + L1 blocks of
                the LAST step. Hoisted into emit_rest(T-2) right after the
                L2 emission so it overlaps the t=T-2 tail instead of
                serializing after it. Touches only vT0/s0-planes (no vT12)."""
                t = T - 1
                s0 = kp.tile([128, H0], F16, tag="s0", name=f"s0_t{t}")
                sA0 = kp.tile([128, H0], E5, tag="sA0", name=f"sA0_t{t}")
                sB0 = kp.tile([128, H0], E5, tag="sB0", name=f"sB0_t{t}")
                sP_cur[0] = (sA0, sB0)
                QH = H0 // 4
                for q in range(4):
                    lif_release0_part(t, q * QH, QH)
                    lif_spike(0, t, s0, q * QH, QH)
                    make_planes_half(s0, sA0, sB0, q * QH, QH)
                    emit_L1(t, kpis=(q,))
                fin0[0] = True

            def emit_rest(t, filler=None):
                last = (t == T - 1)
                if last:
                    if not fin0[0]:
                        emit_final_l0_chain()
                else:
                    s0 = kp.tile([128, H0], F16, tag="s0", name=f"s0_t{t}")
                    sA0 = kp.tile([128, H0], E5, tag="sA0", name=f"sA0_t{t}")
                    sB0 = kp.tile([128, H0], E5, tag="sB0", name=f"sB0_t{t}")
                    sP_cur[0] = (sA0, sB0)
                    lif_spike(0, t, s0, 0, H0)
                    lif_states(0, t, s0, last)
                    lif_pre(0, t + 1)
                    make_planes_half(s0, sA0, sB0, 0, H0)
                    dbg_dump("s0", t, s0[:])
                    dbg_dump("sa0", t, sA0[:])
                    dbg_dump("sb0", t, sB0[:])
                    if t == 0:
                        dbg_dump("b2b1", 0, B2b[1][:])
                        dbg_dump("a1", 0, A_[1][:])
                    emit_L1(t)
                if DBG and t in DBG_T:
                    nc.scalar.copy(dstg[:, :H1], C[1][:])
                    dbg_dump("c1", t, dstg[:, :H1])
                if not last:
                    lif_release(1, t)
                dbg_dump("v1", t, vT[1][:, :H1])
                if filler is not None:
                    filler()
                s1 = kp.tile([128, H1], F16, tag="s1", name=f"s1_t{t}")
                sA1 = kp.tile([128, H1], E5, tag="sA1", name=f"sA1_t{t}")
                sB1 = kp.tile([128, H1], E5, tag="sB1", name=f"sB1_t{t}")
                def emit_L2(kpis):
                    # L2: 3 digit products suffice (CPU-validated 0.0127 with
                    # the full scheme); plane-A products first so the PE has
                    # work before sB1 lands
                    for kpi in kpis:
                        for i in (0, 1, 2):
                            spl = sA1 if i < 2 else sB1
                            rhs = spl[:].rearrange("p (c b) -> p c b", b=BL)[
                                :, 2 * kpi:2 * kpi + 2, :]
                            lhs = wod[:, (kpi * NWD + i) * 2 * OUT:
                                      (kpi * NWD + i + 1) * 2 * OUT].rearrange(
                                "p (two h) -> p two h", two=2)
                            for m in range(MT2):
                                first = (t == 0 and i == 0 and kpi == 0 and
                                         (m * 128) % ZR == 0)
                                lastm = (t == T - 1 and i == 2 and
                                         kpi == KP2 - 1 and m == MT2 - 1)
                                nc.tensor.matmul(
                                    C[2][:, m * 128:(m + 1) * 128],
                                    lhs[:, :, m * 128:(m + 1) * 128], rhs,
                                    start=first, stop=lastm, perf_mode=DR,
                                    skip_group_check=True)
                if last:
                    QH1 = H1 // 4
                    for q in range(4):
                        lif_release(1, t, off=q * QH1, hh=QH1)
                        lif_spike(1, t, s1, q * QH1, QH1)
                        make_planes_half(s1, sA1, sB1, q * QH1, QH1)
                        emit_L2((q,))
                else:
                    lif_spike(1, t, s1, 0, H1)
                    make_planes_half(s1, sA1, sB1, 0, H1)
                    lif_states(1, t, s1, last)
                    lif_pre(1, t + 1)
                    emit_L2(range(KP2))
                if t == T - 2:
                    # hoist the last step's l0 chain over this step's tail
                    emit_final_l0_chain()
                if t == 0:
                    inject_bias(2)
                lif_release(2, t)
                dbg_dump("v2", t, vT[2][:, :OUT])
                nc.vector.scalar_tensor_tensor(
                    out=acc[:], in0=vT[2][:, :OUT], scalar=0.5, in1=acc[:],
                    op0=Alu.is_gt, op1=Alu.add)
                if not last:
                    s2 = kp.tile([128, OUT], F16, tag="s2", name=f"s2_t{t}")
                    lif_spike(2, t, s2, 0, OUT)
                    lif_states(2, t, s2, last)
                    lif_pre(2, t + 1)

            # ---- main loop: 1-step layer skew ----
            for t in range(T):
                if t >= 1:
                    lif_release0(t - 1)   # frees C0A/C0B for step t's matmuls
                emit_L0(t, cis=(0,))
                if t == 0:
                    load_x(1, cis=(0,))
                    dma_w1d(range(1, 3))
                    load_x(1, cis=(1,))
                    dma_w1d(range(3, KP1))
                    dma_wo()
                    build_B2b()
                    for l in (0, 1, 2):
                        lif_pre(l, 0)
                    emit_L0(0, cis=(1,))
                else:
                    emit_rest(t - 1, filler=lambda tt=t: emit_L0(tt, cis=(1,)))
                    if t + 1 < T:
                        load_x(t + 1)
            emit_rest(T - 1)

            nc.sync.dma_start(out=out_d[:], in_=acc[:])

    nc.compile()
    _BUILD_CACHE[key] = nc
    return nc


def _split_f16(a32, lo_scale=2048.0):
    hi = a32.astype(np.float16)
    lo = ((a32 - hi.astype(np.float32)) * np.float32(lo_scale)).astype(np.float16)
    return hi, lo


def _digits32(r, nd):
    """Signed radix-32 digit planes of r (float64, |r|<=0.5): returns list
    of integer-valued planes d_i with r ~= sum d_i * 32^-i, |d_i| <= 16."""
    r = r.copy()
    planes = []
    for i in range(1, nd + 1):
        di = np.rint(r * 32.0 ** i)
        np.clip(di, -16, 16, out=di)
        planes.append(di)
        r = r - di * 32.0 ** -i
    return planes


def _w_planes(WT, scales, check=True):
    """WT [in,out] fp32 -> [in, nd*2*out] e4m3: radix-32 digit planes of
    WT*2^EW, plane i stored *2^scales[i], k-pair interleaved rows.
    Output row r = (kp*128+p) holds planes for in-rows 2*kp*128+p (two=0)
    and (2*kp+1)*128+p (two=1)."""
    IN, OUTD = WT.shape
    nd = len(scales)
    r = WT.astype(np.float64) * (2.0 ** EW)
    assert np.max(np.abs(r)) <= 0.5, "weight prescale EW too small"
    dws = _digits32(r, nd)
    planes = [d * (2.0 ** s) for d, s in zip(dws, scales)]
    # [IN, nd, OUT] -> rows (kp, two, p) -> (kp, p, i, two, out)
    arr = np.stack(planes, axis=1).reshape(IN // 256, 2, 128, nd, OUTD)
    arr = np.transpose(arr, (0, 2, 3, 1, 4))  # kp, p, i, two, out
    out = np.ascontiguousarray(arr).astype(ml_dtypes.float8_e4m3fn)
    if check:
        assert np.all(out.astype(np.float64) == arr), \
            "digit planes not exact in fp8e4"
    return out.reshape(IN // 2, nd * 2 * OUTD)


def _x_planes(x):
    """x [T, IN, B] fp32 -> [T, IN//2, NXD*2*B] e4m3 digit planes of
    (x-0.5), plane j stored *2^XSC[j], rows (kp,p), inner (j, two, b)."""
    Tn, IN, B = x.shape
    out = np.empty((Tn, IN // 256, 128, NXD, 2, B), dtype=ml_dtypes.float8_e4m3fn)
    for t in range(Tn):
        r = x[t].astype(np.float64) - 0.5
        dxs = _digits32(r, NXD)
        planes = [d * (2.0 ** s) for d, s in zip(dxs, XSC)]
        arr = np.stack(planes, axis=1).reshape(IN // 256, 2, 128, NXD, B)
        arr = np.transpose(arr, (0, 2, 3, 1, 4))  # kp, p, j, two, b
        a8 = np.ascontiguousarray(arr).astype(ml_dtypes.float8_e4m3fn)
        if t == 0:
            assert np.all(a8.astype(np.float64) == arr), \
                "x digit planes not exact in fp8e4"
        out[t] = a8
    return out.reshape(Tn, IN // 2, NXD * 2 * B)


def prep_inputs(in_pop_spikes, W0, b0, W1, b1, Wout, bout,
                T=16, BL=128, ncores=NCORES):
    x = np.ascontiguousarray(np.transpose(np.asarray(in_pop_spikes, np.float32),
                                          (2, 1, 0)))  # [T, IN, B]
    xp = _x_planes(x)

    com = {}
    W0T = np.ascontiguousarray(np.asarray(W0, np.float32).T)
    com["w0d"] = _w_planes(W0T, W0SC)
    com["w1d"] = _w_planes(np.ascontiguousarray(np.asarray(W1, np.float32).T),
                           W12SC)
    com["wod"] = _w_planes(np.ascontiguousarray(np.asarray(Wout, np.float32).T),
                           W12SC)
    b0_eff = (np.asarray(b0, np.float64)
              + 0.5 * np.asarray(W0, np.float64).sum(axis=1)).astype(np.float32)
    e = 2.0 ** EL12
    parts = [np.stack([np.full(128, 2.0, np.float16),
                       np.full(128, 2.0 / 2048.0, np.float16)]),
             np.stack([np.full(128, -e, np.float16),
                       np.full(128, -e / 2048.0, np.float16)]),
             np.stack([np.full(128, -e, np.float16),
                       np.full(128, -e / 2048.0, np.float16)])]
    for b in (b0_eff, b1, bout):
        hi, lo = _split_f16(np.asarray(b, np.float32))
        parts.append(np.stack([hi, lo]))
    com["cst"] = np.ascontiguousarray(np.concatenate(parts, axis=1))

    in_maps = []
    for c in range(ncores):
        m = dict(com)
        m["xp"] = np.ascontiguousarray(
            xp.reshape(T, xp.shape[1], NXD * 2, -1)[:, :, :, c * BL:(c + 1) * BL]
            .reshape(T, xp.shape[1], NXD * 2 * BL))
        in_maps.append(m)
    return in_maps


def kernel(in_pop_spikes, W0, b0, W1, b1, Wout, bout, batch_size, _trace=False):
    T = in_pop_spikes.shape[2]
    OUT, BL = Wout.shape[0], 128
    nc = build(**FULL)
    in_maps = prep_inputs(in_pop_spikes, W0, b0, W1, b1, Wout, bout, T=T)
    res = run_bass_kernel_spmd(nc, in_maps, core_ids=list(range(NCORES)),
                               trace=_trace)
    # device out[p, (c,b)] -> [b, c*128+p]
    outs = []
    for r in res.results:
        a = r["out"].reshape(128, OUT // 128, BL)
        outs.append(np.transpose(a, (2, 1, 0)).reshape(BL, OUT))
    out = (np.concatenate(outs, axis=0) / np.float32(T)).astype(np.float32)
    if _trace:
        kernel._last_results = res
    return out
